# revision 3
# baseline (speedup 1.0000x reference)
"""Trainium2 Bass kernel for CoordinationAnalysis (histogram binning).

Math (reference):
  dists[i,j] = ||x_i - x_j||  (diagonal excluded)
  boundaries = softplus(shell_boundaries);  shells = [b_s, b_{s+1})
  occ[i,s]  = #{j : b_s <= d_ij < b_{s+1}} / NORM
  out       = x + occ @ W.T + b,  also return occ

Device formulation (per 128x512 tile of the pairwise-product matrix):
  q_ij = x_i . x_j - ||x_j||^2/2        (via matmul with one augmentation row)
  d_ij < t  <=>  q_ij > r_i^t,  r_i^t = ||x_i||^2/2 - t^2/2
  N_t[i] = #{j : q_ij > r_i^t}  -> occ counts = N_{t+1} - N_t  (diagonal cancels)
Counts are computed with DVE tensor_scalar(is_gt) + accum_out on bf16 tiles
(4x perf mode); matmul runs in bf16 (distances are O(500), thresholds < 5,
so bf16 slack is ~1e2 margin); final feature matmul + x add in fp32.

Sharding: rows of x data-parallel across 8 cores (1024 rows each); x^T
replicated in SBUF of every core.
"""

import sys

sys.path.insert(0, "/opt/trn_rl_repo")

from contextlib import ExitStack

import numpy as np
import ml_dtypes

from concourse import bacc, mybir, tile, bass_utils
from concourse.bass_interp import get_hw_module

N, D, S = 8192, 256, 3
TARGET_COORD = 1.081
N_CORES = 8
NSH = N // N_CORES          # 1024 shard rows per core
IT = NSH // 128             # 8 i-tiles per core
JBLK = 512                  # matmul free dim / PSUM bank
JSUP = 2048                 # DVE compare chunk
NT = S + 1                  # 4 thresholds
F32 = mybir.dt.float32
BF16 = mybir.dt.bfloat16

_cached = {}


def _build():
    nc = bacc.Bacc("TRN2", target_bir_lowering=False, debug=False,
                   num_devices=N_CORES)

    xt = nc.dram_tensor("xt", [2, 128, N], BF16, kind="ExternalInput")
    xtsh = nc.dram_tensor("xtsh", [2, 128, NSH], BF16, kind="ExternalInput")
    nhsq = nc.dram_tensor("nhsq", [1, N], BF16, kind="ExternalInput")
    xsh = nc.dram_tensor("xsh", [128, IT * D], F32, kind="ExternalInput")
    thr = nc.dram_tensor("thr", [128, IT * NT], F32, kind="ExternalInput")
    wb = nc.dram_tensor("wb", [NT, D], F32, kind="ExternalInput")
    eye = nc.dram_tensor("eye", [128, 128], F32, kind="ExternalInput")
    coord = nc.dram_tensor("coord", [NSH, D], F32, kind="ExternalOutput")
    occ = nc.dram_tensor("occ", [NSH, S], F32, kind="ExternalOutput")

    with tile.TileContext(nc) as tc, ExitStack() as ctx:
        persist = ctx.enter_context(tc.tile_pool(name="persist", bufs=1))
        qb_pool = ctx.enter_context(tc.tile_pool(name="qb", bufs=3))
        cnt_pool = ctx.enter_context(tc.tile_pool(name="cnt", bufs=2))
        small = ctx.enter_context(tc.tile_pool(name="small", bufs=2))
        out_pool = ctx.enter_context(tc.tile_pool(name="outp", bufs=2))
        psum_q = ctx.enter_context(tc.tile_pool(name="psq", bufs=4, space="PSUM"))
        psum_t = ctx.enter_context(tc.tile_pool(name="pst", bufs=1, space="PSUM"))
        psum_f = ctx.enter_context(tc.tile_pool(name="psf", bufs=2, space="PSUM"))

        # persistent SBUF residents
        xt_sb = [persist.tile([128, N], BF16, name=f"xt{k}", tag=f"xt{k}")
                 for k in range(2)]
        xtsh_sb = [persist.tile([128, NSH], BF16, name=f"xtsh{k}", tag=f"xtsh{k}")
                   for k in range(2)]
        nhsq_sb = persist.tile([1, N], BF16, tag="nhsq")
        xsh_sb = persist.tile([128, IT * D], F32, tag="xsh")
        thr_sb = persist.tile([128, IT * NT], F32, tag="thr")
        wb_sb = persist.tile([NT, D], F32, tag="wb")
        eye_sb = persist.tile([128, 128], F32, tag="eye")
        ones_sb = persist.tile([1, 128], BF16, tag="ones")
        junk = persist.tile([128, JSUP], BF16, tag="junk")

        for k in range(2):
            nc.sync.dma_start(xtsh_sb[k][:], xtsh.ap()[k])
            # split the big replicated x^T load so early matmuls start sooner
            for cchunk in range(4):
                nc.sync.dma_start(
                    xt_sb[k][:, cchunk * JSUP:(cchunk + 1) * JSUP],
                    xt.ap()[k, :, cchunk * JSUP:(cchunk + 1) * JSUP],
                )
        nc.sync.dma_start(nhsq_sb[:], nhsq.ap()[:])
        nc.sync.dma_start(xsh_sb[:], xsh.ap()[:])
        nc.sync.dma_start(thr_sb[:], thr.ap()[:])
        nc.sync.dma_start(wb_sb[:], wb.ap()[:])
        nc.sync.dma_start(eye_sb[:], eye.ap()[:])
        nc.vector.memset(ones_sb[:], 1.0)

        nsup = N // JSUP  # 4 j-supertiles
        for it in range(IT):
            i0 = it * 128
            cnt_all = cnt_pool.tile([128, NT * nsup], F32)
            for js in range(nsup):
                qb = qb_pool.tile([128, JSUP], BF16)
                for sub in range(JSUP // JBLK):
                    j0 = js * JSUP + sub * JBLK
                    ps = psum_q.tile([128, JBLK], F32)
                    nc.tensor.matmul(
                        ps[:], xtsh_sb[0][:, i0:i0 + 128],
                        xt_sb[0][:, j0:j0 + JBLK], start=True, stop=False)
                    nc.tensor.matmul(
                        ps[:], xtsh_sb[1][:, i0:i0 + 128],
                        xt_sb[1][:, j0:j0 + JBLK], start=False, stop=False)
                    nc.tensor.matmul(
                        ps[:], ones_sb[:1, :], nhsq_sb[:1, j0:j0 + JBLK],
                        start=False, stop=True)
                    nc.scalar.copy(qb[:, sub * JBLK:(sub + 1) * JBLK], ps[:])
                for t in range(NT):
                    nc.vector.tensor_scalar(
                        junk[:], qb[:], thr_sb[:, it * NT + t:it * NT + t + 1],
                        None, mybir.AluOpType.is_gt, op1=mybir.AluOpType.add,
                        accum_out=cnt_all[:, t * nsup + js:t * nsup + js + 1])

            nbuf = small.tile([128, NT], F32, tag="nbuf")
            for t in range(NT):
                nc.vector.tensor_reduce(
                    nbuf[:, t:t + 1], cnt_all[:, t * nsup:(t + 1) * nsup],
                    mybir.AxisListType.X, mybir.AluOpType.add)
            occ4 = small.tile([128, NT], F32, tag="occ4")
            nc.vector.tensor_tensor(
                occ4[:, 0:S], nbuf[:, 1:NT], nbuf[:, 0:S],
                mybir.AluOpType.subtract)
            nc.vector.memset(occ4[:, S:NT], 1.0)

            occo = small.tile([128, S], F32, tag="occo")
            nc.vector.tensor_scalar_mul(occo[:], occ4[:, 0:S],
                                        1.0 / (TARGET_COORD * (N - 1) + 1e-8))
            nc.sync.dma_start(occ.ap()[i0:i0 + 128, :], occo[:])

            pt = psum_t.tile([NT, 128], F32)
            nc.tensor.transpose(pt[:], occ4[:], eye_sb[:])
            cntT = small.tile([NT, 128], F32, tag="cntT")
            nc.scalar.copy(cntT[:], pt[:])
            pf = psum_f.tile([128, D], F32)
            nc.tensor.matmul(pf[:], cntT[:], wb_sb[:], start=True, stop=True)
            outt = out_pool.tile([128, D], F32)
            nc.vector.tensor_tensor(outt[:], pf[:],
                                    xsh_sb[:, it * D:(it + 1) * D],
                                    mybir.AluOpType.add)
            nc.sync.dma_start(coord.ap()[i0:i0 + 128, :], outt[:])

    nc.compile()
    nc.m = get_hw_module(nc.m)
    return nc


def _prep_inputs(x, shell_boundaries, W, b):
    x = np.asarray(x, np.float32)
    sb = np.asarray(shell_boundaries, np.float32)
    W = np.asarray(W, np.float32)
    b = np.asarray(b, np.float32)

    bounds = np.logaddexp(0.0, sb.astype(np.float64))        # softplus
    half_t2 = 0.5 * bounds * bounds                          # [NT]
    sq = np.einsum("nd,nd->n", x.astype(np.float64), x.astype(np.float64))
    thr_full = (0.5 * sq[:, None] - half_t2[None, :]).astype(np.float32)  # [N,4]
    nhsq_full = (-0.5 * sq).astype(np.float32)

    xt_bf = np.ascontiguousarray(x.T).astype(ml_dtypes.bfloat16)  # [256, N]
    xt3 = xt_bf.reshape(2, 128, N)

    norm = TARGET_COORD * (N - 1) + 1e-8
    wb_host = np.concatenate([(W.T / norm).astype(np.float32),
                              b[None, :]], axis=0)           # [4, 256]
    eye_host = np.eye(128, dtype=np.float32)
    nhsq_bf = nhsq_full.astype(ml_dtypes.bfloat16)[None, :]  # [1, N]

    in_maps = []
    for c in range(N_CORES):
        lo, hi = c * NSH, (c + 1) * NSH
        xtsh_c = np.ascontiguousarray(xt3[:, :, lo:hi])
        xsh_c = np.ascontiguousarray(
            x[lo:hi].reshape(IT, 128, D).transpose(1, 0, 2).reshape(128, IT * D))
        thr_c = np.ascontiguousarray(
            thr_full[lo:hi].reshape(IT, 128, NT).transpose(1, 0, 2)
            .reshape(128, IT * NT))
        in_maps.append({
            "xt": xt3, "xtsh": xtsh_c, "nhsq": nhsq_bf, "xsh": xsh_c,
            "thr": thr_c, "wb": wb_host, "eye": eye_host,
        })
    return in_maps


def _install_ntff_hook_shim():
    """The image's antenv package lacks axon_hooks; rebuild the NTFF
    profiling hook from libaxon_pjrt.so and inject the module."""
    import types, ctypes, contextlib

    if "antenv.axon_hooks" in sys.modules:
        return
    so_path = "/opt/axon/libaxon_pjrt.so"
    hook = None
    try:
        lib = ctypes.CDLL(so_path)
        if hasattr(lib, "axon_start_nrt_profile"):
            lib.axon_start_nrt_profile.argtypes = [
                ctypes.POINTER(ctypes.c_int64), ctypes.c_size_t]
            lib.axon_start_nrt_profile.restype = ctypes.c_int64
            lib.axon_stop_nrt_profile.argtypes = [ctypes.c_char_p]
            lib.axon_stop_nrt_profile.restype = ctypes.c_int64

            @contextlib.contextmanager
            def _hook(output_dir, device_ids):
                import jax
                jax.devices()
                if device_ids:
                    ids = (ctypes.c_int64 * len(device_ids))(*device_ids)
                    rc = lib.axon_start_nrt_profile(ids, len(device_ids))
                else:
                    rc = lib.axon_start_nrt_profile(None, 0)
                if rc != 0:
                    raise RuntimeError(f"axon_start_nrt_profile rc={rc}")
                try:
                    yield
                finally:
                    n = lib.axon_stop_nrt_profile(str(output_dir).encode())
                    print(f"profile: {n} file(s) written to {output_dir}",
                          file=sys.stderr)

            hook = _hook
    except OSError:
        pass

    mod = types.ModuleType("antenv.axon_hooks")
    mod.get_axon_ntff_profile_hook = lambda: hook
    mod.set_axon_ntff_profile_hook = lambda h: None
    sys.modules["antenv.axon_hooks"] = mod


def run(x, shell_boundaries, W, b, trace=False, **trace_kwargs):
    if trace:
        _install_ntff_hook_shim()
    if "nc" not in _cached:
        _cached["nc"] = _build()
    nc = _cached["nc"]
    in_maps = _prep_inputs(x, shell_boundaries, W, b)
    res = bass_utils.run_bass_kernel_spmd(
        nc, in_maps, core_ids=list(range(N_CORES)), trace=trace, **trace_kwargs)
    coord = np.concatenate([res.results[c]["coord"] for c in range(N_CORES)], 0)
    occ = np.concatenate([res.results[c]["occ"] for c in range(N_CORES)], 0)
    return (coord.astype(np.float32), occ.astype(np.float32)), res


def kernel(x, shell_boundaries, W, b):
    out, _ = run(x, shell_boundaries, W, b)
    return out


# revision 4
# speedup vs baseline: 1.0080x; 1.0080x over previous
"""Trainium2 Bass kernel for CoordinationAnalysis (histogram binning).

Math (reference):
  dists[i,j] = ||x_i - x_j||  (diagonal excluded)
  boundaries = softplus(shell_boundaries);  shells = [b_s, b_{s+1})
  occ[i,s]  = #{j : b_s <= d_ij < b_{s+1}} / NORM
  out       = x + occ @ W.T + b,  also return occ

Device formulation (per 128x512 tile of the pairwise-product matrix):
  q_ij = x_i . x_j - ||x_j||^2/2        (via matmul with one augmentation row)
  d_ij < t  <=>  q_ij > r_i^t,  r_i^t = ||x_i||^2/2 - t^2/2
  N_t[i] = #{j : q_ij > r_i^t}  -> occ counts = N_{t+1} - N_t  (diagonal cancels)
Counts are computed with DVE tensor_scalar(is_gt) + accum_out on bf16 tiles
(4x perf mode); matmul runs in bf16 (distances are O(500), thresholds < 5,
so bf16 slack is ~1e2 margin); final feature matmul + x add in fp32.

Sharding: rows of x data-parallel across 8 cores (1024 rows each); x^T
replicated in SBUF of every core.
"""

import sys

sys.path.insert(0, "/opt/trn_rl_repo")

from contextlib import ExitStack

import numpy as np
import ml_dtypes

from concourse import bacc, mybir, tile, bass_utils
from concourse.bass_interp import get_hw_module

N, D, S = 8192, 256, 3
TARGET_COORD = 1.081
N_CORES = 8
NSH = N // N_CORES          # 1024 shard rows per core
IT = NSH // 128             # 8 i-tiles per core
JBLK = 512                  # matmul free dim / PSUM bank
JSUP = 2048                 # DVE compare chunk
NT = S + 1                  # 4 thresholds
F32 = mybir.dt.float32
F16 = mybir.dt.float16

_cached = {}


def _build():
    nc = bacc.Bacc("TRN2", target_bir_lowering=False, debug=False,
                   num_devices=N_CORES)

    xt = nc.dram_tensor("xt", [2, 128, N], F16, kind="ExternalInput")
    xtsh = nc.dram_tensor("xtsh", [2, 128, NSH], F16, kind="ExternalInput")
    nhsq = nc.dram_tensor("nhsq", [1, N], F16, kind="ExternalInput")
    xsh = nc.dram_tensor("xsh", [128, IT * D], F32, kind="ExternalInput")
    thr = nc.dram_tensor("thr", [128, IT * NT], F32, kind="ExternalInput")
    wb = nc.dram_tensor("wb", [NT, D], F32, kind="ExternalInput")
    eye = nc.dram_tensor("eye", [128, 128], F32, kind="ExternalInput")
    coord = nc.dram_tensor("coord", [NSH, D], F32, kind="ExternalOutput")
    occ = nc.dram_tensor("occ", [NSH, S], F32, kind="ExternalOutput")

    with tile.TileContext(nc) as tc, ExitStack() as ctx:
        persist = ctx.enter_context(tc.tile_pool(name="persist", bufs=1))
        qb_pool = ctx.enter_context(tc.tile_pool(name="qb", bufs=3))
        cnt_pool = ctx.enter_context(tc.tile_pool(name="cnt", bufs=2))
        small = ctx.enter_context(tc.tile_pool(name="small", bufs=2))
        out_pool = ctx.enter_context(tc.tile_pool(name="outp", bufs=2))
        psum_q = ctx.enter_context(tc.tile_pool(name="psq", bufs=4, space="PSUM"))
        psum_t = ctx.enter_context(tc.tile_pool(name="pst", bufs=1, space="PSUM"))
        psum_f = ctx.enter_context(tc.tile_pool(name="psf", bufs=2, space="PSUM"))

        # persistent SBUF residents
        xt_sb = [persist.tile([128, N], F16, name=f"xt{k}", tag=f"xt{k}")
                 for k in range(2)]
        xtsh_sb = [persist.tile([128, NSH], F16, name=f"xtsh{k}", tag=f"xtsh{k}")
                   for k in range(2)]
        nhsq_sb = persist.tile([1, N], F16, tag="nhsq")
        xsh_sb = persist.tile([128, IT * D], F32, tag="xsh")
        thr_sb = persist.tile([128, IT * NT], F32, tag="thr")
        wb_sb = persist.tile([NT, D], F32, tag="wb")
        eye_sb = persist.tile([128, 128], F32, tag="eye")
        ones_sb = persist.tile([1, 128], F16, tag="ones")
        junk = persist.tile([128, JSUP], F16, tag="junk")

        for k in range(2):
            nc.sync.dma_start(xtsh_sb[k][:], xtsh.ap()[k])
            # split the big replicated x^T load so early matmuls start sooner
            for cchunk in range(4):
                nc.sync.dma_start(
                    xt_sb[k][:, cchunk * JSUP:(cchunk + 1) * JSUP],
                    xt.ap()[k, :, cchunk * JSUP:(cchunk + 1) * JSUP],
                )
        nc.sync.dma_start(nhsq_sb[:], nhsq.ap()[:])
        nc.sync.dma_start(xsh_sb[:], xsh.ap()[:])
        nc.sync.dma_start(thr_sb[:], thr.ap()[:])
        nc.sync.dma_start(wb_sb[:], wb.ap()[:])
        nc.sync.dma_start(eye_sb[:], eye.ap()[:])
        nc.vector.memset(ones_sb[:], 1.0)

        nsup = N // JSUP  # 4 j-supertiles
        for it in range(IT):
            i0 = it * 128
            cnt_all = cnt_pool.tile([128, NT * nsup], F32)
            for js in range(nsup):
                qb = qb_pool.tile([128, JSUP], F16)
                for sub in range(JSUP // JBLK):
                    j0 = js * JSUP + sub * JBLK
                    ps = psum_q.tile([128, JBLK], F32)
                    nc.tensor.matmul(
                        ps[:], xtsh_sb[0][:, i0:i0 + 128],
                        xt_sb[0][:, j0:j0 + JBLK], start=True, stop=False)
                    nc.tensor.matmul(
                        ps[:], xtsh_sb[1][:, i0:i0 + 128],
                        xt_sb[1][:, j0:j0 + JBLK], start=False, stop=False)
                    nc.tensor.matmul(
                        ps[:], ones_sb[:1, :], nhsq_sb[:1, j0:j0 + JBLK],
                        start=False, stop=True)
                    nc.scalar.copy(qb[:, sub * JBLK:(sub + 1) * JBLK], ps[:])
                for t in range(NT):
                    nc.vector.tensor_scalar(
                        junk[:], qb[:], thr_sb[:, it * NT + t:it * NT + t + 1],
                        None, mybir.AluOpType.is_gt, op1=mybir.AluOpType.add,
                        accum_out=cnt_all[:, t * nsup + js:t * nsup + js + 1])

            nbuf = small.tile([128, NT], F32, tag="nbuf")
            for t in range(NT):
                nc.vector.tensor_reduce(
                    nbuf[:, t:t + 1], cnt_all[:, t * nsup:(t + 1) * nsup],
                    mybir.AxisListType.X, mybir.AluOpType.add)
            occ4 = small.tile([128, NT], F32, tag="occ4")
            nc.vector.tensor_tensor(
                occ4[:, 0:S], nbuf[:, 1:NT], nbuf[:, 0:S],
                mybir.AluOpType.subtract)
            nc.vector.memset(occ4[:, S:NT], 1.0)

            occo = small.tile([128, S], F32, tag="occo")
            nc.vector.tensor_scalar_mul(occo[:], occ4[:, 0:S],
                                        1.0 / (TARGET_COORD * (N - 1) + 1e-8))
            nc.sync.dma_start(occ.ap()[i0:i0 + 128, :], occo[:])

            pt = psum_t.tile([NT, 128], F32)
            nc.tensor.transpose(pt[:], occ4[:], eye_sb[:])
            cntT = small.tile([NT, 128], F32, tag="cntT")
            nc.scalar.copy(cntT[:], pt[:])
            pf = psum_f.tile([128, D], F32)
            nc.tensor.matmul(pf[:], cntT[:], wb_sb[:], start=True, stop=True)
            outt = out_pool.tile([128, D], F32)
            nc.vector.tensor_tensor(outt[:], pf[:],
                                    xsh_sb[:, it * D:(it + 1) * D],
                                    mybir.AluOpType.add)
            nc.sync.dma_start(coord.ap()[i0:i0 + 128, :], outt[:])

    nc.compile()
    nc.m = get_hw_module(nc.m)
    return nc


def _prep_inputs(x, shell_boundaries, W, b):
    x = np.asarray(x, np.float32)
    sb = np.asarray(shell_boundaries, np.float32)
    W = np.asarray(W, np.float32)
    b = np.asarray(b, np.float32)

    bounds = np.logaddexp(0.0, sb.astype(np.float64))        # softplus
    half_t2 = 0.5 * bounds * bounds                          # [NT]
    sq = np.einsum("nd,nd->n", x.astype(np.float64), x.astype(np.float64))
    thr_full = (0.5 * sq[:, None] - half_t2[None, :]).astype(np.float32)  # [N,4]
    nhsq_full = (-0.5 * sq).astype(np.float32)

    xt_bf = np.ascontiguousarray(x.T).astype(np.float16)  # [256, N]
    xt3 = xt_bf.reshape(2, 128, N)

    norm = TARGET_COORD * (N - 1) + 1e-8
    wb_host = np.concatenate([(W.T / norm).astype(np.float32),
                              b[None, :]], axis=0)           # [4, 256]
    eye_host = np.eye(128, dtype=np.float32)
    nhsq_bf = nhsq_full.astype(np.float16)[None, :]  # [1, N]

    in_maps = []
    for c in range(N_CORES):
        lo, hi = c * NSH, (c + 1) * NSH
        xtsh_c = np.ascontiguousarray(xt3[:, :, lo:hi])
        xsh_c = np.ascontiguousarray(
            x[lo:hi].reshape(IT, 128, D).transpose(1, 0, 2).reshape(128, IT * D))
        thr_c = np.ascontiguousarray(
            thr_full[lo:hi].reshape(IT, 128, NT).transpose(1, 0, 2)
            .reshape(128, IT * NT))
        in_maps.append({
            "xt": xt3, "xtsh": xtsh_c, "nhsq": nhsq_bf, "xsh": xsh_c,
            "thr": thr_c, "wb": wb_host, "eye": eye_host,
        })
    return in_maps


def _install_ntff_hook_shim():
    """The image's antenv package lacks axon_hooks; rebuild the NTFF
    profiling hook from libaxon_pjrt.so and inject the module."""
    import types, ctypes, contextlib

    if "antenv.axon_hooks" in sys.modules:
        return
    so_path = "/opt/axon/libaxon_pjrt.so"
    hook = None
    try:
        lib = ctypes.CDLL(so_path)
        if hasattr(lib, "axon_start_nrt_profile"):
            lib.axon_start_nrt_profile.argtypes = [
                ctypes.POINTER(ctypes.c_int64), ctypes.c_size_t]
            lib.axon_start_nrt_profile.restype = ctypes.c_int64
            lib.axon_stop_nrt_profile.argtypes = [ctypes.c_char_p]
            lib.axon_stop_nrt_profile.restype = ctypes.c_int64

            @contextlib.contextmanager
            def _hook(output_dir, device_ids):
                import jax
                jax.devices()
                if device_ids:
                    ids = (ctypes.c_int64 * len(device_ids))(*device_ids)
                    rc = lib.axon_start_nrt_profile(ids, len(device_ids))
                else:
                    rc = lib.axon_start_nrt_profile(None, 0)
                if rc != 0:
                    raise RuntimeError(f"axon_start_nrt_profile rc={rc}")
                try:
                    yield
                finally:
                    n = lib.axon_stop_nrt_profile(str(output_dir).encode())
                    print(f"profile: {n} file(s) written to {output_dir}",
                          file=sys.stderr)

            hook = _hook
    except OSError:
        pass

    mod = types.ModuleType("antenv.axon_hooks")
    mod.get_axon_ntff_profile_hook = lambda: hook
    mod.set_axon_ntff_profile_hook = lambda h: None
    sys.modules["antenv.axon_hooks"] = mod


def run(x, shell_boundaries, W, b, trace=False, **trace_kwargs):
    if trace:
        _install_ntff_hook_shim()
    if "nc" not in _cached:
        _cached["nc"] = _build()
    nc = _cached["nc"]
    in_maps = _prep_inputs(x, shell_boundaries, W, b)
    res = bass_utils.run_bass_kernel_spmd(
        nc, in_maps, core_ids=list(range(N_CORES)), trace=trace, **trace_kwargs)
    coord = np.concatenate([res.results[c]["coord"] for c in range(N_CORES)], 0)
    occ = np.concatenate([res.results[c]["occ"] for c in range(N_CORES)], 0)
    return (coord.astype(np.float32), occ.astype(np.float32)), res


def kernel(x, shell_boundaries, W, b):
    out, _ = run(x, shell_boundaries, W, b)
    return out


# revision 10
# speedup vs baseline: 1.1734x; 1.1641x over previous
"""Trainium2 Bass kernel for CoordinationAnalysis (histogram binning).

Math (reference):
  dists[i,j] = ||x_i - x_j||  (diagonal excluded)
  boundaries = softplus(shell_boundaries);  shells = [b_s, b_{s+1})
  occ[i,s]  = #{j : b_s <= d_ij < b_{s+1}} / NORM
  out       = x + occ @ W.T + b,  also return occ

Device formulation (per 128x512 tile of the pairwise-product matrix):
  q_ij = x_i . x_j - ||x_j||^2/2        (via matmul with one augmentation row)
  d_ij < t  <=>  q_ij > r_i^t,  r_i^t = ||x_i||^2/2 - t^2/2
  N_t[i] = #{j : q_ij > r_i^t}  -> occ counts = N_{t+1} - N_t  (diagonal cancels)
Counts are computed with DVE tensor_scalar(is_gt) + accum_out on bf16 tiles
(4x perf mode); matmul runs in bf16 (distances are O(500), thresholds < 5,
so bf16 slack is ~1e2 margin); final feature matmul + x add in fp32.

Sharding: rows of x data-parallel across 8 cores (1024 rows each); x^T
replicated in SBUF of every core.
"""

import sys

sys.path.insert(0, "/opt/trn_rl_repo")

from contextlib import ExitStack

import numpy as np
import ml_dtypes

from concourse import bacc, mybir, tile, bass_utils
from concourse.bass_interp import get_hw_module

N, D, S = 8192, 256, 3
TARGET_COORD = 1.081
N_CORES = 8
NSH = N // N_CORES          # 1024 shard rows per core
IT = NSH // 128             # 8 i-tiles per core
JBLK = 512                  # matmul free dim / PSUM bank
JSUP = 2048                 # DVE compare chunk
NT = S + 1                  # 4 thresholds
F32 = mybir.dt.float32
F16 = mybir.dt.float16

_cached = {}


def _build():
    nc = bacc.Bacc("TRN2", target_bir_lowering=False, debug=False,
                   num_devices=N_CORES)

    xt = nc.dram_tensor("xt", [2, 128, N], F16, kind="ExternalInput")
    xtsh = nc.dram_tensor("xtsh", [2, 128, NSH], F16, kind="ExternalInput")
    nhsq = nc.dram_tensor("nhsq", [1, N], F16, kind="ExternalInput")
    xsh = nc.dram_tensor("xsh", [128, IT * D], F32, kind="ExternalInput")
    thr = nc.dram_tensor("thr", [128, IT * NT], F32, kind="ExternalInput")
    thrn = nc.dram_tensor("thrn", [128, IT * NT], F32, kind="ExternalInput")
    wb = nc.dram_tensor("wb", [NT, D], F32, kind="ExternalInput")
    eye = nc.dram_tensor("eye", [128, 128], F32, kind="ExternalInput")
    coord = nc.dram_tensor("coord", [NSH, D], F32, kind="ExternalOutput")
    occ = nc.dram_tensor("occ", [NSH, S], F32, kind="ExternalOutput")

    with tile.TileContext(nc) as tc, ExitStack() as ctx:
        persist = ctx.enter_context(tc.tile_pool(name="persist", bufs=1))
        small = ctx.enter_context(tc.tile_pool(name="small", bufs=2))
        out_pool = ctx.enter_context(tc.tile_pool(name="outp", bufs=2))

        # persistent SBUF residents
        xt_sb = [persist.tile([128, N], F16, name=f"xt{k}", tag=f"xt{k}")
                 for k in range(2)]
        xtsh_sb = [persist.tile([128, NSH], F16, name=f"xtsh{k}", tag=f"xtsh{k}")
                   for k in range(2)]
        nhsq_sb = persist.tile([1, N], F16, tag="nhsq")
        xsh_sb = persist.tile([128, IT * D], F32, tag="xsh")
        thr_sb = persist.tile([128, IT * NT], F32, tag="thr")
        thrn_sb = persist.tile([128, IT * NT], F32, tag="thrn")
        wb_sb = persist.tile([NT, D], F32, tag="wb")
        eye_sb = persist.tile([128, 128], F32, tag="eye")
        ones_sb = persist.tile([1, 128], F16, tag="ones")
        junk_d = persist.tile([128, JSUP], F16, tag="junkd")
        junk_a = persist.tile([128, JSUP], F16, tag="junka")
        # raw counts (DVE, thresholds 0-1) and sign-sums (ACT, thresholds 2-3),
        # one column per j-supertile, all i-tiles
        cnt_d = persist.tile([128, IT * 2 * 4], F32, tag="cntd")
        cnt_a = persist.tile([128, IT * 2 * 4], F32, tag="cnta")

        for k in range(2):
            nc.sync.dma_start(xtsh_sb[k][:], xtsh.ap()[k])
            # split the big replicated x^T load so early matmuls start sooner
            for cchunk in range(4):
                nc.sync.dma_start(
                    xt_sb[k][:, cchunk * JSUP:(cchunk + 1) * JSUP],
                    xt.ap()[k, :, cchunk * JSUP:(cchunk + 1) * JSUP],
                )
        nc.sync.dma_start(nhsq_sb[:], nhsq.ap()[:])
        nc.sync.dma_start(xsh_sb[:], xsh.ap()[:])
        nc.sync.dma_start(thr_sb[:], thr.ap()[:])
        nc.sync.dma_start(thrn_sb[:], thrn.ap()[:])
        nc.sync.dma_start(wb_sb[:], wb.ap()[:])
        nc.sync.dma_start(eye_sb[:], eye.ap()[:])
        nc.vector.memset(ones_sb[:], 1.0)

        nsup = N // JSUP  # 4 j-supertiles
        GT = mybir.AluOpType.is_gt
        ADD = mybir.AluOpType.add

        # Phase 1: pairwise products in PSUM, counts read PSUM directly.
        # DVE takes thresholds 0-1 (is_gt + accum), ACT takes 2-3 via
        # Sign(q - r) + accum (count = 0.5*sum + N/2).
        with tc.tile_pool(name="psq", bufs=2, space="PSUM") as psum_q:
            for it in range(IT):
                i0 = it * 128
                for js in range(nsup):
                    ps = psum_q.tile([128, JSUP], F32)
                    for sub in range(JSUP // JBLK):
                        j0 = js * JSUP + sub * JBLK
                        sl = ps[:, sub * JBLK:(sub + 1) * JBLK]
                        nc.tensor.matmul(
                            sl, xtsh_sb[0][:, i0:i0 + 128],
                            xt_sb[0][:, j0:j0 + JBLK], start=True, stop=False)
                        nc.tensor.matmul(
                            sl, xtsh_sb[1][:, i0:i0 + 128],
                            xt_sb[1][:, j0:j0 + JBLK], start=False, stop=False)
                        nc.tensor.matmul(
                            sl, ones_sb[:1, :], nhsq_sb[:1, j0:j0 + JBLK],
                            start=False, stop=True)
                    for tl in range(2):
                        col = (it * 2 + tl) * 4 + js
                        nc.vector.tensor_scalar(
                            junk_d[:], ps[:],
                            thr_sb[:, it * NT + tl:it * NT + tl + 1],
                            None, GT, op1=ADD,
                            accum_out=cnt_d[:, col:col + 1])
                    for tl in range(2):
                        t = 2 + tl
                        col = (it * 2 + tl) * 4 + js
                        nc.scalar.activation(
                            junk_a[:], ps[:],
                            mybir.ActivationFunctionType.Sign,
                            bias=thrn_sb[:, it * NT + t:it * NT + t + 1],
                            accum_out=cnt_a[:, col:col + 1])

        # Phase 2: fold partial counts, normalize, shell_embed + residual.
        with tc.tile_pool(name="pse", bufs=2, space="PSUM") as psum_e:
            for it in range(IT):
                i0 = it * 128
                nbuf = small.tile([128, NT], F32, tag="nbuf")
                for tl in range(2):
                    c0 = (it * 2 + tl) * 4
                    nc.vector.tensor_reduce(
                        nbuf[:, tl:tl + 1], cnt_d[:, c0:c0 + 4],
                        mybir.AxisListType.X, ADD)
                for tl in range(2):
                    c0 = (it * 2 + tl) * 4
                    nc.vector.tensor_reduce(
                        nbuf[:, 2 + tl:3 + tl], cnt_a[:, c0:c0 + 4],
                        mybir.AxisListType.X, ADD)
                nc.vector.tensor_scalar(
                    nbuf[:, 2:NT], nbuf[:, 2:NT], 0.5, float(N) / 2.0,
                    mybir.AluOpType.mult, op1=ADD)
                occ4 = small.tile([128, NT], F32, tag="occ4")
                nc.vector.tensor_tensor(
                    occ4[:, 0:S], nbuf[:, 1:NT], nbuf[:, 0:S],
                    mybir.AluOpType.subtract)
                nc.vector.memset(occ4[:, S:NT], 1.0)

                occo = small.tile([128, S], F32, tag="occo")
                nc.vector.tensor_scalar_mul(
                    occo[:], occ4[:, 0:S],
                    1.0 / (TARGET_COORD * (N - 1) + 1e-8))
                nc.sync.dma_start(occ.ap()[i0:i0 + 128, :], occo[:])

                pt = psum_e.tile([NT, 128], F32, tag="pt")
                nc.tensor.transpose(pt[:], occ4[:], eye_sb[:])
                cntT = small.tile([NT, 128], F32, tag="cntT")
                nc.scalar.copy(cntT[:], pt[:])
                pf = psum_e.tile([128, D], F32, tag="pf")
                nc.tensor.matmul(pf[:], cntT[:], wb_sb[:], start=True, stop=True)
                outt = out_pool.tile([128, D], F32)
                nc.vector.tensor_tensor(outt[:], pf[:],
                                        xsh_sb[:, it * D:(it + 1) * D],
                                        ADD)
                nc.sync.dma_start(coord.ap()[i0:i0 + 128, :], outt[:])

    nc.compile()
    nc.m = get_hw_module(nc.m)
    return nc


def _prep_inputs(x, shell_boundaries, W, b):
    x = np.asarray(x, np.float32)
    sb = np.asarray(shell_boundaries, np.float32)
    W = np.asarray(W, np.float32)
    b = np.asarray(b, np.float32)

    bounds = np.logaddexp(0.0, sb.astype(np.float64))        # softplus
    half_t2 = 0.5 * bounds * bounds                          # [NT]
    sq = np.einsum("nd,nd->n", x.astype(np.float64), x.astype(np.float64))
    thr_full = (0.5 * sq[:, None] - half_t2[None, :]).astype(np.float32)  # [N,4]
    nhsq_full = (-0.5 * sq).astype(np.float32)

    xt_bf = np.ascontiguousarray(x.T).astype(np.float16)  # [256, N]
    xt3 = xt_bf.reshape(2, 128, N)

    norm = TARGET_COORD * (N - 1) + 1e-8
    wb_host = np.concatenate([(W.T / norm).astype(np.float32),
                              b[None, :]], axis=0)           # [4, 256]
    eye_host = np.eye(128, dtype=np.float32)
    nhsq_bf = nhsq_full.astype(np.float16)[None, :]  # [1, N]

    in_maps = []
    for c in range(N_CORES):
        lo, hi = c * NSH, (c + 1) * NSH
        xtsh_c = np.ascontiguousarray(xt3[:, :, lo:hi])
        xsh_c = np.ascontiguousarray(
            x[lo:hi].reshape(IT, 128, D).transpose(1, 0, 2).reshape(128, IT * D))
        thr_c = np.ascontiguousarray(
            thr_full[lo:hi].reshape(IT, 128, NT).transpose(1, 0, 2)
            .reshape(128, IT * NT))
        in_maps.append({
            "xt": xt3, "xtsh": xtsh_c, "nhsq": nhsq_bf, "xsh": xsh_c,
            "thr": thr_c, "thrn": np.ascontiguousarray(-thr_c),
            "wb": wb_host, "eye": eye_host,
        })
    return in_maps


def _install_ntff_hook_shim():
    """The image's antenv package lacks axon_hooks; rebuild the NTFF
    profiling hook from libaxon_pjrt.so and inject the module."""
    import types, ctypes, contextlib

    if "antenv.axon_hooks" in sys.modules:
        return
    so_path = "/opt/axon/libaxon_pjrt.so"
    hook = None
    try:
        lib = ctypes.CDLL(so_path)
        if hasattr(lib, "axon_start_nrt_profile"):
            lib.axon_start_nrt_profile.argtypes = [
                ctypes.POINTER(ctypes.c_int64), ctypes.c_size_t]
            lib.axon_start_nrt_profile.restype = ctypes.c_int64
            lib.axon_stop_nrt_profile.argtypes = [ctypes.c_char_p]
            lib.axon_stop_nrt_profile.restype = ctypes.c_int64

            @contextlib.contextmanager
            def _hook(output_dir, device_ids):
                import jax
                jax.devices()
                if device_ids:
                    ids = (ctypes.c_int64 * len(device_ids))(*device_ids)
                    rc = lib.axon_start_nrt_profile(ids, len(device_ids))
                else:
                    rc = lib.axon_start_nrt_profile(None, 0)
                if rc != 0:
                    raise RuntimeError(f"axon_start_nrt_profile rc={rc}")
                try:
                    yield
                finally:
                    n = lib.axon_stop_nrt_profile(str(output_dir).encode())
                    print(f"profile: {n} file(s) written to {output_dir}",
                          file=sys.stderr)

            hook = _hook
    except OSError:
        pass

    mod = types.ModuleType("antenv.axon_hooks")
    mod.get_axon_ntff_profile_hook = lambda: hook
    mod.set_axon_ntff_profile_hook = lambda h: None
    sys.modules["antenv.axon_hooks"] = mod


def run(x, shell_boundaries, W, b, trace=False, **trace_kwargs):
    if trace:
        _install_ntff_hook_shim()
    if "nc" not in _cached:
        _cached["nc"] = _build()
    nc = _cached["nc"]
    in_maps = _prep_inputs(x, shell_boundaries, W, b)
    res = bass_utils.run_bass_kernel_spmd(
        nc, in_maps, core_ids=list(range(N_CORES)), trace=trace, **trace_kwargs)
    coord = np.concatenate([res.results[c]["coord"] for c in range(N_CORES)], 0)
    occ = np.concatenate([res.results[c]["occ"] for c in range(N_CORES)], 0)
    return (coord.astype(np.float32), occ.astype(np.float32)), res


def kernel(x, shell_boundaries, W, b):
    out, _ = run(x, shell_boundaries, W, b)
    return out
